# revision 6
# baseline (speedup 1.0000x reference)
"""Trainium2 Bass kernel for the DAN classifier (gather + segment-mean + MLP + BCE).

Data-parallel over 8 cores (512 sentences each). Host sorts each core's
tokens by (96-seg group, vocab-quarter, 32-seg band, vocab) and pads each
(group, quarter) bucket to a shared tile count. On device, one dma_gather
per bucket streams bf16 embedding rows over 4 SWDGE queues (emission
strictly rotates queues so Tile's round-robin DMASW lanes stay
queue-locked). Segment sums accumulate on the TensorEngine via narrow
32-seg one-hot matmuls (host-known band->tile ranges) into one PSUM
accumulation group per seg-group, bracketed by full-width start/stop
matmuls; quarters accumulate into the same bank. MLP head + BCE on-chip;
host sums the 8 partial losses.
"""

import sys

try:
    import concourse  # noqa: F401
except ImportError:
    sys.path.insert(0, "/opt/trn_rl_repo")

import ml_dtypes
import numpy as np

import concourse.tile as tile
from concourse import bacc, mybir
from concourse.bass_utils import run_bass_kernel_spmd

V = 100000
H = 128
B = 4096
T = 409600
N_CORES = 8

SEGS_PER_CORE = B // N_CORES          # 512
GROUP_SEGS = (96, 96, 96, 96, 96, 32)
GROUP_STARTS = (0, 96, 192, 288, 384, 480)
N_GROUPS = len(GROUP_SEGS)
N_QUARTERS = 4
VQ = 25600
BAND = 32

F32 = mybir.dt.float32
BF16 = mybir.dt.bfloat16
I16 = mybir.dt.int16
BF16_NP = ml_dtypes.bfloat16


def _build(nc, struct):
    """struct: tuple of per-bucket (t_gq, ((band, lo, hi), ...)) in
    (g, q) order; all compile-time constants shared across cores."""
    n_buckets = N_GROUPS * N_QUARTERS
    tiles = [s[0] for s in struct]
    t_max = max(tiles)
    tot_tiles = sum(tiles)
    col16 = [t * 8 for t in tiles]          # idx cols (128 idx = 8 cols)
    col16_off = np.cumsum([0] + col16).tolist()
    tile_off = np.cumsum([0] + tiles).tolist()

    embed = nc.dram_tensor("embed", [V, H], BF16, kind="ExternalInput")
    idx_d = nc.dram_tensor("idx", [128, col16_off[-1]], I16,
                           kind="ExternalInput")
    seg_d = nc.dram_tensor("seg", [128, tot_tiles], BF16,
                           kind="ExternalInput")
    recip_d = nc.dram_tensor("recip", [128, N_GROUPS], F32,
                             kind="ExternalInput")
    iota_d = nc.dram_tensor("iota", [128, 128], BF16, kind="ExternalInput")
    ident_d = nc.dram_tensor("ident", [128, 128], F32, kind="ExternalInput")
    y_d = nc.dram_tensor("y", [1, SEGS_PER_CORE], F32, kind="ExternalInput")
    w_hid_d = nc.dram_tensor("w_hid", [H, H], F32, kind="ExternalInput")
    b_hid_d = nc.dram_tensor("b_hid", [H, 1], F32, kind="ExternalInput")
    w_out_d = nc.dram_tensor("w_out", [H, 1], F32, kind="ExternalInput")
    b_out_d = nc.dram_tensor("b_out", [1, 1], F32, kind="ExternalInput")
    out_d = nc.dram_tensor("out", [1, 1], F32, kind="ExternalOutput")

    with tile.TileContext(nc) as tc:
        with (
            tc.tile_pool(name="const", bufs=1) as cpool,
            tc.tile_pool(name="gather", bufs=4) as gpool,
            tc.tile_pool(name="onehot", bufs=4) as opool,
            tc.tile_pool(name="mlp", bufs=1) as mpool,
            tc.tile_pool(name="psum_seg", bufs=3, space="PSUM") as pspool,
            tc.tile_pool(name="psum_t", bufs=2, space="PSUM") as ptpool,
            tc.tile_pool(name="psum_mlp", bufs=1, space="PSUM") as pmpool,
        ):
            # ---- idx loads: group 0 in one transfer, rest in another ----
            idx_sb = cpool.tile([128, col16_off[-1]], I16)
            g0_hi = col16_off[N_QUARTERS]
            nc.sync.dma_start(out=idx_sb[:, :g0_hi], in_=idx_d[:, :g0_hi])
            nc.sync.dma_start(out=idx_sb[:, g0_hi:], in_=idx_d[:, g0_hi:])
            seg_sb = cpool.tile([128, tot_tiles], BF16)
            nc.scalar.dma_start(out=seg_sb[:], in_=seg_d[:])
            iota_sb = cpool.tile([128, 128], BF16)
            nc.scalar.dma_start(out=iota_sb[:], in_=iota_d[:])
            recip_sb = cpool.tile([128, N_GROUPS], F32)
            nc.scalar.dma_start(out=recip_sb[:], in_=recip_d[:])
            ident_sb = cpool.tile([128, 128], F32)
            nc.scalar.dma_start(out=ident_sb[:], in_=ident_d[:])
            y_sb = cpool.tile([1, SEGS_PER_CORE], F32)
            nc.scalar.dma_start(out=y_sb[:], in_=y_d[:])
            w_hid_sb = cpool.tile([H, H], F32)
            nc.scalar.dma_start(out=w_hid_sb[:], in_=w_hid_d[:])
            b_hid_sb = cpool.tile([H, 1], F32)
            nc.scalar.dma_start(out=b_hid_sb[:], in_=b_hid_d[:])
            w_out_sb = cpool.tile([H, 1], F32)
            nc.scalar.dma_start(out=w_out_sb[:], in_=w_out_d[:])
            b_out_sb = cpool.tile([1, 1], F32)
            nc.scalar.dma_start(out=b_out_sb[:], in_=b_out_d[:])

            # ---- gather preps (pipelined: gen overlaps previous DMA) ----
            # sub-calls of <=1024 descriptors to stay within the SWDGE
            # descriptor-ring carveout (prepared descs can't be reclaimed
            # until triggered). Emission strictly rotates queues 0..3 so the
            # Tile scheduler's round-robin DMASW lane assignment (8 lanes)
            # keeps each lane locked to a single queue; tile counts are
            # equalized across quarters per group to preserve the rotation.
            gts = {}
            for g in range(N_GROUPS):
                for q in range(N_QUARTERS):
                    sb = g * N_QUARTERS + q
                    gt = gpool.tile([128, t_max, H], BF16, tag=f"g{q}")
                    gts[sb] = gt
                # last two groups: split each bucket into two calls so
                # their matmuls can start half a bucket earlier (queue
                # rotation preserved: every queue emits the same number of
                # calls per round)
                n_parts = 2 if g >= N_GROUPS - 2 else 1
                for part in range(n_parts):
                    for q in range(N_QUARTERS):
                        sb = g * N_QUARTERS + q
                        t_gq = tiles[sb]
                        k0 = (t_gq * part) // n_parts
                        k1 = (t_gq * (part + 1)) // n_parts
                        if k1 == k0:
                            k1 = k0 + 1 if k0 < t_gq else k1
                        nidx = (k1 - k0) * 128
                        c16 = col16_off[sb] + k0 * 8
                        nc.gpsimd.dma_gather(
                            gts[sb][:, k0:k1, :],
                            embed[q * VQ: q * VQ + min(VQ, V - q * VQ), :],
                            idx_sb[:, c16: c16 + nidx // 16],
                            nidx, nidx, H,
                            single_packet=False, queue_num=q)

            # activation table warmups (before first tanh, off critical path)
            warm_act = cpool.tile([1, 1], F32)
            nc.vector.memset(warm_act[:], 0.5)
            for fn in (mybir.ActivationFunctionType.Tanh,
                       mybir.ActivationFunctionType.Exp,
                       mybir.ActivationFunctionType.Ln):
                nc.scalar.activation(out=warm_act[:], in_=warm_act[:], func=fn)

            # ---- per-group segment-sum matmuls + MLP pipeline ----
            sent_t = mpool.tile([128, SEGS_PER_CORE], F32)   # [H, seg]
            psum_hid = pmpool.tile([128, SEGS_PER_CORE], F32, tag="psum_hid")
            hid = mpool.tile([128, SEGS_PER_CORE], F32)
            psum_p = pmpool.tile([1, SEGS_PER_CORE], F32, tag="psum_p")
            ep = mpool.tile([1, SEGS_PER_CORE], F32)
            sp = mpool.tile([1, SEGS_PER_CORE], F32)
            sp_sums = mpool.tile([1, N_GROUPS], F32)
            x_sb = mpool.tile([1, SEGS_PER_CORE], F32)
            yx = mpool.tile([1, SEGS_PER_CORE], F32)

            for g in range(N_GROUPS):
                psum_g = pspool.tile([128, H], F32, tag="seg")
                # One PSUM accumulation group per seg-group: the first matmul
                # (q=0, tile 0) uses a full 128-wide one-hot with start=True
                # (zeroes every partition of the bank region); all later
                # matmuls accumulate 32-seg band slices; the global last one
                # stops the group. PE is in-order, so the sequence is exact.
                t_q3 = tiles[g * N_QUARTERS + 3]

                def _full_mm(q, j, start, stop):
                    sb_ = g * N_QUARTERS + q
                    ohf = opool.tile([128, 1, 128], BF16, tag="ohf")
                    c0 = tile_off[sb_] + j
                    nc.vector.tensor_tensor(
                        out=ohf[:],
                        in0=seg_sb[:, c0: c0 + 1]
                        .rearrange("p (t u) -> p t u", u=1)
                        .to_broadcast([128, 1, 128]),
                        in1=iota_sb[:]
                        .rearrange("p (u m) -> p u m", u=1)
                        .to_broadcast([128, 1, 128]),
                        op=mybir.AluOpType.is_equal,
                    )
                    nc.tensor.matmul(
                        psum_g[:], lhsT=ohf[:, 0, :], rhs=gts[sb_][:, j, :],
                        start=start, stop=stop)

                def _skip(q, j):
                    return (q == 0 and j == 0) or (q == 3 and j == t_q3 - 1)

                # mm0: full-width start (zeroes the whole bank region)
                _full_mm(0, 0, True, False)
                for q in range(N_QUARTERS):
                    sb = g * N_QUARTERS + q
                    gt = gts[sb]
                    # one one-hot batch per (bucket, band) tile range
                    ohs = {}
                    for (b, lo, hi) in struct[sb][1]:
                        nt = hi - lo
                        oh = opool.tile([128, t_max, BAND], BF16, tag="oh")
                        c0 = tile_off[sb] + lo
                        nc.vector.tensor_tensor(
                            out=oh[:, :nt, :],
                            in0=seg_sb[:, c0: c0 + nt]
                            .rearrange("p (t u) -> p t u", u=1)
                            .to_broadcast([128, nt, BAND]),
                            in1=iota_sb[:, b * BAND: (b + 1) * BAND]
                            .rearrange("p (u m) -> p u m", u=1)
                            .to_broadcast([128, nt, BAND]),
                            op=mybir.AluOpType.is_equal,
                        )
                        ohs[b] = (oh, lo)
                    for (b, lo, hi) in struct[sb][1]:
                        oh, lo0 = ohs[b]
                        for j in range(lo, hi):
                            if _skip(q, j):
                                continue
                            nc.tensor.matmul(
                                psum_g[b * BAND: (b + 1) * BAND, :],
                                lhsT=oh[:, j - lo0, :],
                                rhs=gt[:, j, :],
                                start=False, stop=False)
                # mmN: full-width stop (closes the group over the full region)
                _full_mm(3, t_q3 - 1, False, True)

                # segment means for this group
                gstart, gsize = GROUP_STARTS[g], GROUP_SEGS[g]
                sent_g = mpool.tile([128, H], F32, tag="sent_g")
                nc.vector.tensor_scalar(
                    out=sent_g[:gsize, :], in0=psum_g[:gsize, :],
                    scalar1=recip_sb[:gsize, g: g + 1], scalar2=None,
                    op0=mybir.AluOpType.mult)
                psum_t = ptpool.tile([128, 128], F32, tag="psum_t")
                nc.tensor.transpose(psum_t[:], sent_g[:], ident_sb[:])
                nc.vector.tensor_copy(
                    out=sent_t[:, gstart: gstart + gsize],
                    in_=psum_t[:, :gsize])
                nc.tensor.matmul(psum_hid[:, gstart: gstart + gsize],
                                 lhsT=w_hid_sb[:],
                                 rhs=sent_t[:, gstart: gstart + gsize],
                                 start=True, stop=True)
                nc.scalar.activation(
                    out=hid[:, gstart: gstart + gsize],
                    in_=psum_hid[:, gstart: gstart + gsize],
                    func=mybir.ActivationFunctionType.Tanh,
                    bias=b_hid_sb[:, 0:1])
                # head + BCE partials for this group (hidden under gathers)
                gsl = slice(gstart, gstart + gsize)
                nc.tensor.matmul(psum_p[:, gsl], lhsT=w_out_sb[:],
                                 rhs=hid[:, gsl], start=True, stop=True)
                nc.scalar.activation(
                    out=ep[:, gsl], in_=psum_p[:, gsl],
                    func=mybir.ActivationFunctionType.Exp,
                    bias=b_out_sb[0:1, 0:1])
                nc.scalar.activation(
                    out=sp[:, gsl], in_=ep[:, gsl],
                    func=mybir.ActivationFunctionType.Ln,
                    bias=1.0, accum_out=sp_sums[:, g: g + 1])
                nc.vector.tensor_scalar(
                    out=x_sb[:, gsl], in0=psum_p[:, gsl],
                    scalar1=b_out_sb[0:1, 0:1],
                    scalar2=None, op0=mybir.AluOpType.add)
                nc.vector.tensor_tensor(out=yx[:, gsl], in0=y_sb[:, gsl],
                                        in1=x_sb[:, gsl],
                                        op=mybir.AluOpType.mult)

            # ---- final reduction tail ----
            yx_sum = mpool.tile([1, 1], F32)
            nc.vector.tensor_reduce(out=yx_sum[:], in_=yx[:],
                                    axis=mybir.AxisListType.X,
                                    op=mybir.AluOpType.add)
            sp_tot = mpool.tile([1, 1], F32)
            nc.vector.tensor_reduce(out=sp_tot[:], in_=sp_sums[:],
                                    axis=mybir.AxisListType.X,
                                    op=mybir.AluOpType.add)
            loss = mpool.tile([1, 1], F32)
            nc.vector.tensor_tensor(out=loss[:], in0=sp_tot[:], in1=yx_sum[:],
                                    op=mybir.AluOpType.subtract)
            nc.sync.dma_start(out=out_d[:], in_=loss[:])

    nc.compile()
    return nc


def _prep_inputs(token_ids, segment_ids, y_true, embed_table, W_hid, b_hid,
                 W_out, b_out):
    token_ids = np.asarray(token_ids, dtype=np.int64)
    segment_ids = np.asarray(segment_ids, dtype=np.int64)
    y_true = np.asarray(y_true, dtype=np.float32)
    embed_bf16 = np.ascontiguousarray(
        np.asarray(embed_table, dtype=np.float32).astype(BF16_NP))

    bounds = np.searchsorted(segment_ids, np.arange(0, B + 1, SEGS_PER_CORE))
    counts = np.bincount(segment_ids, minlength=B).astype(np.float32)
    recip_all = 1.0 / np.maximum(counts, 1.0)

    starts = np.asarray(GROUP_STARTS, dtype=np.int64)
    n_buckets = N_GROUPS * N_QUARTERS

    # per (core, bucket): loc-sorted-by-(band, vocab) indices + seg + band
    per_core = []
    for c in range(N_CORES):
        lo, hi = bounds[c], bounds[c + 1]
        tid = token_ids[lo:hi]
        seg_loc = segment_ids[lo:hi] - c * SEGS_PER_CORE
        grp = np.searchsorted(starts[1:], seg_loc, side="right")
        sig = seg_loc - starts[grp]
        band = sig // BAND
        q = tid // VQ
        locv = tid - q * VQ
        order = np.lexsort((locv, band, q, grp))
        grp_s, q_s = grp[order], q[order]
        li_s, sg_s, bd_s = locv[order], sig[order], band[order]
        bucket = grp_s * N_QUARTERS + q_s
        bnd = np.searchsorted(bucket, np.arange(n_buckets + 1))
        subs = []
        for sb in range(n_buckets):
            s, e = bnd[sb], bnd[sb + 1]
            subs.append((li_s[s:e].astype(np.int64),
                         sg_s[s:e].astype(np.float32),
                         bd_s[s:e].astype(np.int64)))
        per_core.append(subs)

    # uniform tile counts across cores (one gather call per bucket keeps
    # the queue rotation strict regardless of per-quarter tile counts) +
    # band->tile ranges (union over cores)
    tiles = []
    band_ranges = []
    for sb in range(n_buckets):
        t_gq = max(max((len(per_core[c][sb][0]) + 127) // 128, 1)
                   for c in range(N_CORES))
        tiles.append(t_gq)
        g = sb // N_QUARTERS
        nb = (GROUP_SEGS[g] + BAND - 1) // BAND
        rng = {}
        for c in range(N_CORES):
            bd = per_core[c][sb][2]
            for b in range(nb):
                pos = np.flatnonzero(bd == b)
                if pos.size == 0:
                    continue
                lo_t, hi_t = pos[0] // 128, pos[-1] // 128 + 1
                if b in rng:
                    rng[b] = (min(rng[b][0], lo_t), max(rng[b][1], hi_t))
                else:
                    rng[b] = (lo_t, hi_t)
        band_ranges.append(tuple((b, lo, hi) for b, (lo, hi)
                                 in sorted(rng.items())))
    struct = tuple((tiles[sb], band_ranges[sb]) for sb in range(n_buckets))

    tile_off = np.cumsum([0] + tiles).tolist()
    col16 = [t * 8 for t in tiles]
    col16_off = np.cumsum([0] + col16).tolist()
    tot_tiles = tile_off[-1]

    iota = np.broadcast_to(np.arange(128, dtype=np.float32),
                           (128, 128)).astype(BF16_NP)
    in_maps = []
    for c in range(N_CORES):
        idx_arr = np.zeros((128, col16_off[-1]), dtype=np.int16)
        seg_arr = np.full((128, tot_tiles), -1.0, dtype=BF16_NP)
        for sb in range(n_buckets):
            li, sg, _ = per_core[c][sb]
            n = li.shape[0]
            nidx = tiles[sb] * 128
            ip = np.zeros(nidx, dtype=np.int16)
            ip[:n] = li
            if n:
                ip[n:] = li[-1]          # pad: repeat last row (seg=-1)
            sp = np.full(nidx, -1.0, dtype=np.float32)
            sp[:n] = sg
            wrapped = ip.reshape(nidx // 16, 16).T
            idx_arr[:, col16_off[sb]: col16_off[sb + 1]] = np.tile(
                wrapped, (8, 1))
            seg_arr[:, tile_off[sb]: tile_off[sb + 1]] = (
                sp.reshape(tiles[sb], 128).T.astype(BF16_NP))
        recip_c = np.ones((128, N_GROUPS), dtype=np.float32)
        for g in range(N_GROUPS):
            gstart, gsize = GROUP_STARTS[g], GROUP_SEGS[g]
            recip_c[:gsize, g] = recip_all[
                c * SEGS_PER_CORE + gstart: c * SEGS_PER_CORE + gstart + gsize]
        in_maps.append({
            "embed": embed_bf16,
            "idx": idx_arr,
            "seg": seg_arr,
            "recip": recip_c,
            "iota": iota,
            "ident": np.eye(128, dtype=np.float32),
            "y": np.ascontiguousarray(
                y_true[c * SEGS_PER_CORE:(c + 1) * SEGS_PER_CORE]
            ).reshape(1, SEGS_PER_CORE),
            "w_hid": np.ascontiguousarray(np.asarray(W_hid, dtype=np.float32)),
            "b_hid": np.asarray(b_hid, dtype=np.float32).reshape(H, 1),
            "w_out": np.ascontiguousarray(np.asarray(W_out, dtype=np.float32)),
            "b_out": np.asarray(b_out, dtype=np.float32).reshape(1, 1),
        })
    return struct, in_maps


_CACHE = {}


def _get_nc(struct):
    nc = _CACHE.get(struct)
    if nc is None:
        nc = bacc.Bacc("TRN2", target_bir_lowering=False, debug=False,
                       num_devices=N_CORES, num_swdge_queues=N_QUARTERS)
        _build(nc, struct)
        _CACHE[struct] = nc
    return nc


def kernel(token_ids, segment_ids, y_true, embed_table, W_hid, b_hid, W_out,
           b_out, _trace=False, _trace_kwargs=None):
    struct, in_maps = _prep_inputs(token_ids, segment_ids, y_true,
                                   embed_table, W_hid, b_hid, W_out, b_out)
    nc = _get_nc(struct)
    res = run_bass_kernel_spmd(nc, in_maps, core_ids=list(range(N_CORES)),
                               trace=_trace, **(_trace_kwargs or {}))
    total = np.float64(0.0)
    for r in res.results:
        total += np.float64(r["out"][0, 0])
    out = np.array(np.float32(total))
    if _trace:
        return out, res
    return out


# revision 7
# speedup vs baseline: 1.0036x; 1.0036x over previous
"""Trainium2 Bass kernel for the DAN classifier (gather + segment-mean + MLP + BCE).

Data-parallel over 8 cores (512 sentences each). Host sorts each core's
tokens by (96-seg group, vocab-quarter, 32-seg band, vocab) and pads each
(group, quarter) bucket to a shared tile count. On device, one dma_gather
per bucket streams bf16 embedding rows over 4 SWDGE queues (emission
strictly rotates queues so Tile's round-robin DMASW lanes stay
queue-locked). Segment sums accumulate on the TensorEngine via narrow
32-seg one-hot matmuls (host-known band->tile ranges) into one PSUM
accumulation group per seg-group, bracketed by full-width start/stop
matmuls; quarters accumulate into the same bank. MLP head + BCE on-chip;
host sums the 8 partial losses.
"""

import sys

try:
    import concourse  # noqa: F401
except ImportError:
    sys.path.insert(0, "/opt/trn_rl_repo")

import ml_dtypes
import numpy as np

import concourse.tile as tile
from concourse import bacc, mybir
from concourse.bass_utils import run_bass_kernel_spmd

V = 100000
H = 128
B = 4096
T = 409600
N_CORES = 8

SEGS_PER_CORE = B // N_CORES          # 512
GROUP_SEGS = (96, 96, 96, 96, 96, 32)
GROUP_STARTS = (0, 96, 192, 288, 384, 480)
N_GROUPS = len(GROUP_SEGS)
N_QUARTERS = 4
VQ = 25600
BAND = 32

F32 = mybir.dt.float32
BF16 = mybir.dt.bfloat16
I16 = mybir.dt.int16
BF16_NP = ml_dtypes.bfloat16


def _build(nc, struct):
    """struct: tuple of per-bucket (t_gq, ((band, lo, hi), ...)) in
    (g, q) order; all compile-time constants shared across cores."""
    n_buckets = N_GROUPS * N_QUARTERS
    tiles = [s[0] for s in struct]
    t_max = max(tiles)
    tot_tiles = sum(tiles)
    col16 = [t * 8 for t in tiles]          # idx cols (128 idx = 8 cols)
    col16_off = np.cumsum([0] + col16).tolist()
    tile_off = np.cumsum([0] + tiles).tolist()

    embed = nc.dram_tensor("embed", [V, H], BF16, kind="ExternalInput")
    idx_d = nc.dram_tensor("idx", [128, col16_off[-1]], I16,
                           kind="ExternalInput")
    seg_d = nc.dram_tensor("seg", [128, tot_tiles], BF16,
                           kind="ExternalInput")
    recip_d = nc.dram_tensor("recip", [128, N_GROUPS], F32,
                             kind="ExternalInput")
    iota_d = nc.dram_tensor("iota", [128, 128], BF16, kind="ExternalInput")
    ident_d = nc.dram_tensor("ident", [128, 128], F32, kind="ExternalInput")
    y_d = nc.dram_tensor("y", [1, SEGS_PER_CORE], F32, kind="ExternalInput")
    w_hid_d = nc.dram_tensor("w_hid", [H, H], F32, kind="ExternalInput")
    b_hid_d = nc.dram_tensor("b_hid", [H, 1], F32, kind="ExternalInput")
    w_out_d = nc.dram_tensor("w_out", [H, 1], F32, kind="ExternalInput")
    b_out_d = nc.dram_tensor("b_out", [1, 1], F32, kind="ExternalInput")
    out_d = nc.dram_tensor("out", [1, 1], F32, kind="ExternalOutput")

    with tile.TileContext(nc) as tc:
        with (
            tc.tile_pool(name="const", bufs=1) as cpool,
            tc.tile_pool(name="gather", bufs=4) as gpool,
            tc.tile_pool(name="onehot", bufs=4) as opool,
            tc.tile_pool(name="mlp", bufs=1) as mpool,
            tc.tile_pool(name="psum_seg", bufs=3, space="PSUM") as pspool,
            tc.tile_pool(name="psum_t", bufs=2, space="PSUM") as ptpool,
            tc.tile_pool(name="psum_mlp", bufs=1, space="PSUM") as pmpool,
        ):
            # ---- idx loads: group 0 in one transfer, rest in another ----
            idx_sb = cpool.tile([128, col16_off[-1]], I16)
            g0_hi = col16_off[N_QUARTERS]
            nc.sync.dma_start(out=idx_sb[:, :g0_hi], in_=idx_d[:, :g0_hi])
            nc.sync.dma_start(out=idx_sb[:, g0_hi:], in_=idx_d[:, g0_hi:])
            seg_sb = cpool.tile([128, tot_tiles], BF16)
            nc.scalar.dma_start(out=seg_sb[:], in_=seg_d[:])
            iota_sb = cpool.tile([128, 128], BF16)
            nc.scalar.dma_start(out=iota_sb[:], in_=iota_d[:])
            recip_sb = cpool.tile([128, N_GROUPS], F32)
            nc.scalar.dma_start(out=recip_sb[:], in_=recip_d[:])
            ident_sb = cpool.tile([128, 128], F32)
            nc.scalar.dma_start(out=ident_sb[:], in_=ident_d[:])
            y_sb = cpool.tile([1, SEGS_PER_CORE], F32)
            nc.scalar.dma_start(out=y_sb[:], in_=y_d[:])
            w_hid_sb = cpool.tile([H, H], F32)
            nc.scalar.dma_start(out=w_hid_sb[:], in_=w_hid_d[:])
            b_hid_sb = cpool.tile([H, 1], F32)
            nc.scalar.dma_start(out=b_hid_sb[:], in_=b_hid_d[:])
            w_out_sb = cpool.tile([H, 1], F32)
            nc.scalar.dma_start(out=w_out_sb[:], in_=w_out_d[:])
            b_out_sb = cpool.tile([1, 1], F32)
            nc.scalar.dma_start(out=b_out_sb[:], in_=b_out_d[:])

            # ---- gather preps (pipelined: gen overlaps previous DMA) ----
            # sub-calls of <=1024 descriptors to stay within the SWDGE
            # descriptor-ring carveout (prepared descs can't be reclaimed
            # until triggered). Emission strictly rotates queues 0..3 so the
            # Tile scheduler's round-robin DMASW lane assignment (8 lanes)
            # keeps each lane locked to a single queue; tile counts are
            # equalized across quarters per group to preserve the rotation.
            gts = {}
            for g in range(N_GROUPS):
                for q in range(N_QUARTERS):
                    sb = g * N_QUARTERS + q
                    gt = gpool.tile([128, t_max, H], BF16, tag=f"g{q}")
                    gts[sb] = gt
                for q in range(N_QUARTERS):
                    sb = g * N_QUARTERS + q
                    t_gq = tiles[sb]
                    nidx = t_gq * 128
                    c16 = col16_off[sb]
                    nc.gpsimd.dma_gather(
                        gts[sb][:, :t_gq, :],
                        embed[q * VQ: q * VQ + min(VQ, V - q * VQ), :],
                        idx_sb[:, c16: c16 + nidx // 16],
                        nidx, nidx, H,
                        single_packet=False, queue_num=q)

            # activation table warmups (before first tanh, off critical path)
            warm_act = cpool.tile([1, 1], F32)
            nc.vector.memset(warm_act[:], 0.5)
            for fn in (mybir.ActivationFunctionType.Tanh,
                       mybir.ActivationFunctionType.Exp,
                       mybir.ActivationFunctionType.Ln):
                nc.scalar.activation(out=warm_act[:], in_=warm_act[:], func=fn)

            # ---- per-group segment-sum matmuls + MLP pipeline ----
            sent_t = mpool.tile([128, SEGS_PER_CORE], F32)   # [H, seg]
            psum_hid = pmpool.tile([128, SEGS_PER_CORE], F32, tag="psum_hid")
            hid = mpool.tile([128, SEGS_PER_CORE], F32)
            psum_p = pmpool.tile([1, SEGS_PER_CORE], F32, tag="psum_p")
            ep = mpool.tile([1, SEGS_PER_CORE], F32)
            sp = mpool.tile([1, SEGS_PER_CORE], F32)
            sp_sums = mpool.tile([1, N_GROUPS], F32)
            x_sb = mpool.tile([1, SEGS_PER_CORE], F32)
            yx = mpool.tile([1, SEGS_PER_CORE], F32)

            for g in range(N_GROUPS):
                psum_g = pspool.tile([128, H], F32, tag="seg")
                # One PSUM accumulation group per seg-group: the first matmul
                # (q=0, tile 0) uses a full 128-wide one-hot with start=True
                # (zeroes every partition of the bank region); all later
                # matmuls accumulate 32-seg band slices; the global last one
                # stops the group. PE is in-order, so the sequence is exact.
                t_q3 = tiles[g * N_QUARTERS + 3]

                def _full_mm(q, j, start, stop):
                    sb_ = g * N_QUARTERS + q
                    ohf = opool.tile([128, 1, 128], BF16, tag="ohf")
                    c0 = tile_off[sb_] + j
                    nc.vector.tensor_tensor(
                        out=ohf[:],
                        in0=seg_sb[:, c0: c0 + 1]
                        .rearrange("p (t u) -> p t u", u=1)
                        .to_broadcast([128, 1, 128]),
                        in1=iota_sb[:]
                        .rearrange("p (u m) -> p u m", u=1)
                        .to_broadcast([128, 1, 128]),
                        op=mybir.AluOpType.is_equal,
                    )
                    nc.tensor.matmul(
                        psum_g[:], lhsT=ohf[:, 0, :], rhs=gts[sb_][:, j, :],
                        start=start, stop=stop)

                def _skip(q, j):
                    return (q == 0 and j == 0) or (q == 3 and j == t_q3 - 1)

                # mm0: full-width start (zeroes the whole bank region)
                _full_mm(0, 0, True, False)
                for q in range(N_QUARTERS):
                    sb = g * N_QUARTERS + q
                    gt = gts[sb]
                    # one one-hot batch per (bucket, band) tile range
                    ohs = {}
                    for (b, lo, hi) in struct[sb][1]:
                        nt = hi - lo
                        oh = opool.tile([128, t_max, BAND], BF16, tag="oh")
                        c0 = tile_off[sb] + lo
                        nc.vector.tensor_tensor(
                            out=oh[:, :nt, :],
                            in0=seg_sb[:, c0: c0 + nt]
                            .rearrange("p (t u) -> p t u", u=1)
                            .to_broadcast([128, nt, BAND]),
                            in1=iota_sb[:, b * BAND: (b + 1) * BAND]
                            .rearrange("p (u m) -> p u m", u=1)
                            .to_broadcast([128, nt, BAND]),
                            op=mybir.AluOpType.is_equal,
                        )
                        ohs[b] = (oh, lo)
                    for (b, lo, hi) in struct[sb][1]:
                        oh, lo0 = ohs[b]
                        for j in range(lo, hi):
                            if _skip(q, j):
                                continue
                            nc.tensor.matmul(
                                psum_g[b * BAND: (b + 1) * BAND, :],
                                lhsT=oh[:, j - lo0, :],
                                rhs=gt[:, j, :],
                                start=False, stop=False)
                # mmN: full-width stop (closes the group over the full region)
                _full_mm(3, t_q3 - 1, False, True)

                # segment means for this group
                gstart, gsize = GROUP_STARTS[g], GROUP_SEGS[g]
                sent_g = mpool.tile([128, H], F32, tag="sent_g")
                nc.vector.tensor_scalar(
                    out=sent_g[:gsize, :], in0=psum_g[:gsize, :],
                    scalar1=recip_sb[:gsize, g: g + 1], scalar2=None,
                    op0=mybir.AluOpType.mult)
                psum_t = ptpool.tile([128, 128], F32, tag="psum_t")
                nc.tensor.transpose(psum_t[:], sent_g[:], ident_sb[:])
                nc.vector.tensor_copy(
                    out=sent_t[:, gstart: gstart + gsize],
                    in_=psum_t[:, :gsize])
                nc.tensor.matmul(psum_hid[:, gstart: gstart + gsize],
                                 lhsT=w_hid_sb[:],
                                 rhs=sent_t[:, gstart: gstart + gsize],
                                 start=True, stop=True)
                nc.scalar.activation(
                    out=hid[:, gstart: gstart + gsize],
                    in_=psum_hid[:, gstart: gstart + gsize],
                    func=mybir.ActivationFunctionType.Tanh,
                    bias=b_hid_sb[:, 0:1])
                # head + BCE partials for this group (hidden under gathers)
                gsl = slice(gstart, gstart + gsize)
                nc.tensor.matmul(psum_p[:, gsl], lhsT=w_out_sb[:],
                                 rhs=hid[:, gsl], start=True, stop=True)
                nc.scalar.activation(
                    out=ep[:, gsl], in_=psum_p[:, gsl],
                    func=mybir.ActivationFunctionType.Exp,
                    bias=b_out_sb[0:1, 0:1])
                nc.scalar.activation(
                    out=sp[:, gsl], in_=ep[:, gsl],
                    func=mybir.ActivationFunctionType.Ln,
                    bias=1.0, accum_out=sp_sums[:, g: g + 1])
                nc.vector.tensor_scalar(
                    out=x_sb[:, gsl], in0=psum_p[:, gsl],
                    scalar1=b_out_sb[0:1, 0:1],
                    scalar2=None, op0=mybir.AluOpType.add)
                nc.vector.tensor_tensor(out=yx[:, gsl], in0=y_sb[:, gsl],
                                        in1=x_sb[:, gsl],
                                        op=mybir.AluOpType.mult)

            # ---- final reduction tail ----
            yx_sum = mpool.tile([1, 1], F32)
            nc.vector.tensor_reduce(out=yx_sum[:], in_=yx[:],
                                    axis=mybir.AxisListType.X,
                                    op=mybir.AluOpType.add)
            sp_tot = mpool.tile([1, 1], F32)
            nc.vector.tensor_reduce(out=sp_tot[:], in_=sp_sums[:],
                                    axis=mybir.AxisListType.X,
                                    op=mybir.AluOpType.add)
            loss = mpool.tile([1, 1], F32)
            nc.vector.tensor_tensor(out=loss[:], in0=sp_tot[:], in1=yx_sum[:],
                                    op=mybir.AluOpType.subtract)
            nc.sync.dma_start(out=out_d[:], in_=loss[:])

    nc.compile()
    return nc


def _prep_inputs(token_ids, segment_ids, y_true, embed_table, W_hid, b_hid,
                 W_out, b_out):
    token_ids = np.asarray(token_ids, dtype=np.int64)
    segment_ids = np.asarray(segment_ids, dtype=np.int64)
    y_true = np.asarray(y_true, dtype=np.float32)
    embed_bf16 = np.ascontiguousarray(
        np.asarray(embed_table, dtype=np.float32).astype(BF16_NP))

    bounds = np.searchsorted(segment_ids, np.arange(0, B + 1, SEGS_PER_CORE))
    counts = np.bincount(segment_ids, minlength=B).astype(np.float32)
    recip_all = 1.0 / np.maximum(counts, 1.0)

    starts = np.asarray(GROUP_STARTS, dtype=np.int64)
    n_buckets = N_GROUPS * N_QUARTERS

    # per (core, bucket): loc-sorted-by-(band, vocab) indices + seg + band
    per_core = []
    for c in range(N_CORES):
        lo, hi = bounds[c], bounds[c + 1]
        tid = token_ids[lo:hi]
        seg_loc = segment_ids[lo:hi] - c * SEGS_PER_CORE
        grp = np.searchsorted(starts[1:], seg_loc, side="right")
        sig = seg_loc - starts[grp]
        band = sig // BAND
        q = tid // VQ
        locv = tid - q * VQ
        order = np.lexsort((locv, band, q, grp))
        grp_s, q_s = grp[order], q[order]
        li_s, sg_s, bd_s = locv[order], sig[order], band[order]
        bucket = grp_s * N_QUARTERS + q_s
        bnd = np.searchsorted(bucket, np.arange(n_buckets + 1))
        subs = []
        for sb in range(n_buckets):
            s, e = bnd[sb], bnd[sb + 1]
            subs.append((li_s[s:e].astype(np.int64),
                         sg_s[s:e].astype(np.float32),
                         bd_s[s:e].astype(np.int64)))
        per_core.append(subs)

    # uniform tile counts across cores (one gather call per bucket keeps
    # the queue rotation strict regardless of per-quarter tile counts) +
    # band->tile ranges (union over cores)
    tiles = []
    band_ranges = []
    for sb in range(n_buckets):
        t_gq = max(max((len(per_core[c][sb][0]) + 127) // 128, 1)
                   for c in range(N_CORES))
        tiles.append(t_gq)
        g = sb // N_QUARTERS
        nb = (GROUP_SEGS[g] + BAND - 1) // BAND
        rng = {}
        for c in range(N_CORES):
            bd = per_core[c][sb][2]
            for b in range(nb):
                pos = np.flatnonzero(bd == b)
                if pos.size == 0:
                    continue
                lo_t, hi_t = pos[0] // 128, pos[-1] // 128 + 1
                if b in rng:
                    rng[b] = (min(rng[b][0], lo_t), max(rng[b][1], hi_t))
                else:
                    rng[b] = (lo_t, hi_t)
        band_ranges.append(tuple((b, lo, hi) for b, (lo, hi)
                                 in sorted(rng.items())))
    struct = tuple((tiles[sb], band_ranges[sb]) for sb in range(n_buckets))

    tile_off = np.cumsum([0] + tiles).tolist()
    col16 = [t * 8 for t in tiles]
    col16_off = np.cumsum([0] + col16).tolist()
    tot_tiles = tile_off[-1]

    iota = np.broadcast_to(np.arange(128, dtype=np.float32),
                           (128, 128)).astype(BF16_NP)
    in_maps = []
    for c in range(N_CORES):
        idx_arr = np.zeros((128, col16_off[-1]), dtype=np.int16)
        seg_arr = np.full((128, tot_tiles), -1.0, dtype=BF16_NP)
        for sb in range(n_buckets):
            li, sg, _ = per_core[c][sb]
            n = li.shape[0]
            nidx = tiles[sb] * 128
            ip = np.zeros(nidx, dtype=np.int16)
            ip[:n] = li
            if n:
                ip[n:] = li[-1]          # pad: repeat last row (seg=-1)
            sp = np.full(nidx, -1.0, dtype=np.float32)
            sp[:n] = sg
            wrapped = ip.reshape(nidx // 16, 16).T
            idx_arr[:, col16_off[sb]: col16_off[sb + 1]] = np.tile(
                wrapped, (8, 1))
            seg_arr[:, tile_off[sb]: tile_off[sb + 1]] = (
                sp.reshape(tiles[sb], 128).T.astype(BF16_NP))
        recip_c = np.ones((128, N_GROUPS), dtype=np.float32)
        for g in range(N_GROUPS):
            gstart, gsize = GROUP_STARTS[g], GROUP_SEGS[g]
            recip_c[:gsize, g] = recip_all[
                c * SEGS_PER_CORE + gstart: c * SEGS_PER_CORE + gstart + gsize]
        in_maps.append({
            "embed": embed_bf16,
            "idx": idx_arr,
            "seg": seg_arr,
            "recip": recip_c,
            "iota": iota,
            "ident": np.eye(128, dtype=np.float32),
            "y": np.ascontiguousarray(
                y_true[c * SEGS_PER_CORE:(c + 1) * SEGS_PER_CORE]
            ).reshape(1, SEGS_PER_CORE),
            "w_hid": np.ascontiguousarray(np.asarray(W_hid, dtype=np.float32)),
            "b_hid": np.asarray(b_hid, dtype=np.float32).reshape(H, 1),
            "w_out": np.ascontiguousarray(np.asarray(W_out, dtype=np.float32)),
            "b_out": np.asarray(b_out, dtype=np.float32).reshape(1, 1),
        })
    return struct, in_maps


_CACHE = {}


def _get_nc(struct):
    nc = _CACHE.get(struct)
    if nc is None:
        nc = bacc.Bacc("TRN2", target_bir_lowering=False, debug=False,
                       num_devices=N_CORES, num_swdge_queues=N_QUARTERS)
        _build(nc, struct)
        _CACHE[struct] = nc
    return nc


def kernel(token_ids, segment_ids, y_true, embed_table, W_hid, b_hid, W_out,
           b_out, _trace=False, _trace_kwargs=None):
    struct, in_maps = _prep_inputs(token_ids, segment_ids, y_true,
                                   embed_table, W_hid, b_hid, W_out, b_out)
    nc = _get_nc(struct)
    res = run_bass_kernel_spmd(nc, in_maps, core_ids=list(range(N_CORES)),
                               trace=_trace, **(_trace_kwargs or {}))
    total = np.float64(0.0)
    for r in res.results:
        total += np.float64(r["out"][0, 0])
    out = np.array(np.float32(total))
    if _trace:
        return out, res
    return out


# revision 8
# speedup vs baseline: 1.0050x; 1.0014x over previous
"""Trainium2 Bass kernel for the DAN classifier (gather + segment-mean + MLP + BCE).

Data-parallel over 8 cores (512 sentences each). Host sorts each core's
tokens by (96-seg group, vocab-quarter, 32-seg band, vocab) and pads each
(group, quarter) bucket to a shared tile count. On device, one dma_gather
per bucket streams bf16 embedding rows over 4 SWDGE queues (emission
strictly rotates queues so Tile's round-robin DMASW lanes stay
queue-locked). Segment sums accumulate on the TensorEngine via narrow
32-seg one-hot matmuls (host-known band->tile ranges) into one PSUM
accumulation group per seg-group, bracketed by full-width start/stop
matmuls; quarters accumulate into the same bank. MLP head + BCE on-chip;
host sums the 8 partial losses.
"""

import sys

try:
    import concourse  # noqa: F401
except ImportError:
    sys.path.insert(0, "/opt/trn_rl_repo")

import ml_dtypes
import numpy as np

import concourse.tile as tile
from concourse import bacc, mybir
from concourse.bass_utils import run_bass_kernel_spmd

V = 100000
H = 128
B = 4096
T = 409600
N_CORES = 8

SEGS_PER_CORE = B // N_CORES          # 512
GROUP_SEGS = (96, 96, 96, 96, 96, 32)
GROUP_STARTS = (0, 96, 192, 288, 384, 480)
N_GROUPS = len(GROUP_SEGS)
N_QUARTERS = 4
VQ = 25600
BAND = 32

F32 = mybir.dt.float32
BF16 = mybir.dt.bfloat16
I16 = mybir.dt.int16
BF16_NP = ml_dtypes.bfloat16


def _build(nc, struct):
    """struct: tuple of per-bucket (t_gq, ((band, lo, hi), ...)) in
    (g, q) order; all compile-time constants shared across cores."""
    n_buckets = N_GROUPS * N_QUARTERS
    tiles = [s[0] for s in struct]
    t_max = max(tiles)
    tot_tiles = sum(tiles)
    col16 = [t * 8 for t in tiles]          # idx cols (128 idx = 8 cols)
    col16_off = np.cumsum([0] + col16).tolist()
    tile_off = np.cumsum([0] + tiles).tolist()

    embed = nc.dram_tensor("embed", [V, H], BF16, kind="ExternalInput")
    idx_d = nc.dram_tensor("idx", [128, col16_off[-1]], I16,
                           kind="ExternalInput")
    seg_d = nc.dram_tensor("seg", [128, tot_tiles], BF16,
                           kind="ExternalInput")
    recip_d = nc.dram_tensor("recip", [128, N_GROUPS], F32,
                             kind="ExternalInput")
    iota_d = nc.dram_tensor("iota", [128, 128], BF16, kind="ExternalInput")
    ident_d = nc.dram_tensor("ident", [128, 128], F32, kind="ExternalInput")
    y_d = nc.dram_tensor("y", [1, SEGS_PER_CORE], F32, kind="ExternalInput")
    w_hid_d = nc.dram_tensor("w_hid", [H, H], F32, kind="ExternalInput")
    b_hid_d = nc.dram_tensor("b_hid", [H, 1], F32, kind="ExternalInput")
    w_out_d = nc.dram_tensor("w_out", [H, 1], F32, kind="ExternalInput")
    b_out_d = nc.dram_tensor("b_out", [1, 1], F32, kind="ExternalInput")
    out_d = nc.dram_tensor("out", [1, 1], F32, kind="ExternalOutput")

    with tile.TileContext(nc) as tc:
        with (
            tc.tile_pool(name="const", bufs=1) as cpool,
            tc.tile_pool(name="gather", bufs=4) as gpool,
            tc.tile_pool(name="onehot", bufs=4) as opool,
            tc.tile_pool(name="mlp", bufs=1) as mpool,
            tc.tile_pool(name="psum_seg", bufs=3, space="PSUM") as pspool,
            tc.tile_pool(name="psum_t", bufs=2, space="PSUM") as ptpool,
            tc.tile_pool(name="psum_mlp", bufs=1, space="PSUM") as pmpool,
        ):
            # ---- idx loads: group 0 in one transfer, rest in another ----
            idx_sb = cpool.tile([128, col16_off[-1]], I16)
            g0_hi = col16_off[N_QUARTERS]
            nc.sync.dma_start(out=idx_sb[:, :g0_hi], in_=idx_d[:, :g0_hi])
            nc.sync.dma_start(out=idx_sb[:, g0_hi:], in_=idx_d[:, g0_hi:])
            seg_sb = cpool.tile([128, tot_tiles], BF16)
            nc.scalar.dma_start(out=seg_sb[:], in_=seg_d[:])
            iota_sb = cpool.tile([128, 128], BF16)
            nc.scalar.dma_start(out=iota_sb[:], in_=iota_d[:])
            recip_sb = cpool.tile([128, N_GROUPS], F32)
            nc.scalar.dma_start(out=recip_sb[:], in_=recip_d[:])
            ident_sb = cpool.tile([128, 128], F32)
            nc.scalar.dma_start(out=ident_sb[:], in_=ident_d[:])
            y_sb = cpool.tile([1, SEGS_PER_CORE], F32)
            nc.scalar.dma_start(out=y_sb[:], in_=y_d[:])
            w_hid_sb = cpool.tile([H, H], F32)
            nc.scalar.dma_start(out=w_hid_sb[:], in_=w_hid_d[:])
            b_hid_sb = cpool.tile([H, 1], F32)
            nc.scalar.dma_start(out=b_hid_sb[:], in_=b_hid_d[:])
            w_out_sb = cpool.tile([H, 1], F32)
            nc.scalar.dma_start(out=w_out_sb[:], in_=w_out_d[:])
            b_out_sb = cpool.tile([1, 1], F32)
            nc.scalar.dma_start(out=b_out_sb[:], in_=b_out_d[:])

            # ---- gather preps (pipelined: gen overlaps previous DMA) ----
            # sub-calls of <=1024 descriptors to stay within the SWDGE
            # descriptor-ring carveout (prepared descs can't be reclaimed
            # until triggered). Emission strictly rotates queues 0..3 so the
            # Tile scheduler's round-robin DMASW lane assignment (8 lanes)
            # keeps each lane locked to a single queue; tile counts are
            # equalized across quarters per group to preserve the rotation.
            gts = {}
            for g in range(N_GROUPS):
                for q in range(N_QUARTERS):
                    sb = g * N_QUARTERS + q
                    gt = gpool.tile([128, t_max, H], BF16, tag=f"g{q}")
                    gts[sb] = gt
                for q in range(N_QUARTERS):
                    sb = g * N_QUARTERS + q
                    t_gq = tiles[sb]
                    nidx = t_gq * 128
                    c16 = col16_off[sb]
                    nc.gpsimd.dma_gather(
                        gts[sb][:, :t_gq, :],
                        embed[q * VQ: q * VQ + min(VQ, V - q * VQ), :],
                        idx_sb[:, c16: c16 + nidx // 16],
                        nidx, nidx, H,
                        single_packet=False, queue_num=q)

            # activation table warmups (before first tanh, off critical path)
            warm_act = cpool.tile([1, 1], F32)
            nc.vector.memset(warm_act[:], 0.5)
            for fn in (mybir.ActivationFunctionType.Tanh,
                       mybir.ActivationFunctionType.Exp,
                       mybir.ActivationFunctionType.Ln):
                nc.scalar.activation(out=warm_act[:], in_=warm_act[:], func=fn)

            # ---- per-group segment-sum matmuls + MLP pipeline ----
            sent_t = mpool.tile([128, SEGS_PER_CORE], F32)   # [H, seg]
            psum_hid = pmpool.tile([128, SEGS_PER_CORE], F32, tag="psum_hid")
            hid = mpool.tile([128, SEGS_PER_CORE], F32)
            psum_p = pmpool.tile([1, SEGS_PER_CORE], F32, tag="psum_p")
            ep = mpool.tile([1, SEGS_PER_CORE], F32)
            sp = mpool.tile([1, SEGS_PER_CORE], F32)
            sp_sums = mpool.tile([1, N_GROUPS], F32)
            x_sb = mpool.tile([1, SEGS_PER_CORE], F32)
            yx = mpool.tile([1, SEGS_PER_CORE], F32)

            for g in range(N_GROUPS):
                if g == N_GROUPS - 1:
                    # keep the PE p-state up while the last (small) group's
                    # gather streams in: without this the PE idles ~7us, the
                    # clock drops, and the tail matmuls run ~3.7x slower.
                    psum_w = pmpool.tile([128, H], F32, tag="psum_warm")
                    for _ in range(36):
                        nc.tensor.matmul(psum_w[:], lhsT=iota_sb[:],
                                         rhs=iota_sb[:], start=True,
                                         stop=True)
                psum_g = pspool.tile([128, H], F32, tag="seg")
                # One PSUM accumulation group per seg-group: the first matmul
                # (q=0, tile 0) uses a full 128-wide one-hot with start=True
                # (zeroes every partition of the bank region); all later
                # matmuls accumulate 32-seg band slices; the global last one
                # stops the group. PE is in-order, so the sequence is exact.
                t_q3 = tiles[g * N_QUARTERS + 3]

                def _full_mm(q, j, start, stop):
                    sb_ = g * N_QUARTERS + q
                    ohf = opool.tile([128, 1, 128], BF16, tag="ohf")
                    c0 = tile_off[sb_] + j
                    nc.vector.tensor_tensor(
                        out=ohf[:],
                        in0=seg_sb[:, c0: c0 + 1]
                        .rearrange("p (t u) -> p t u", u=1)
                        .to_broadcast([128, 1, 128]),
                        in1=iota_sb[:]
                        .rearrange("p (u m) -> p u m", u=1)
                        .to_broadcast([128, 1, 128]),
                        op=mybir.AluOpType.is_equal,
                    )
                    nc.tensor.matmul(
                        psum_g[:], lhsT=ohf[:, 0, :], rhs=gts[sb_][:, j, :],
                        start=start, stop=stop)

                def _skip(q, j):
                    return (q == 0 and j == 0) or (q == 3 and j == t_q3 - 1)

                # mm0: full-width start (zeroes the whole bank region)
                _full_mm(0, 0, True, False)
                for q in range(N_QUARTERS):
                    sb = g * N_QUARTERS + q
                    gt = gts[sb]
                    # one one-hot batch per (bucket, band) tile range
                    ohs = {}
                    for (b, lo, hi) in struct[sb][1]:
                        nt = hi - lo
                        oh = opool.tile([128, t_max, BAND], BF16, tag="oh")
                        c0 = tile_off[sb] + lo
                        nc.vector.tensor_tensor(
                            out=oh[:, :nt, :],
                            in0=seg_sb[:, c0: c0 + nt]
                            .rearrange("p (t u) -> p t u", u=1)
                            .to_broadcast([128, nt, BAND]),
                            in1=iota_sb[:, b * BAND: (b + 1) * BAND]
                            .rearrange("p (u m) -> p u m", u=1)
                            .to_broadcast([128, nt, BAND]),
                            op=mybir.AluOpType.is_equal,
                        )
                        ohs[b] = (oh, lo)
                    for (b, lo, hi) in struct[sb][1]:
                        oh, lo0 = ohs[b]
                        for j in range(lo, hi):
                            if _skip(q, j):
                                continue
                            nc.tensor.matmul(
                                psum_g[b * BAND: (b + 1) * BAND, :],
                                lhsT=oh[:, j - lo0, :],
                                rhs=gt[:, j, :],
                                start=False, stop=False)
                # mmN: full-width stop (closes the group over the full region)
                _full_mm(3, t_q3 - 1, False, True)

                # segment means for this group
                gstart, gsize = GROUP_STARTS[g], GROUP_SEGS[g]
                sent_g = mpool.tile([128, H], F32, tag="sent_g")
                nc.vector.tensor_scalar(
                    out=sent_g[:gsize, :], in0=psum_g[:gsize, :],
                    scalar1=recip_sb[:gsize, g: g + 1], scalar2=None,
                    op0=mybir.AluOpType.mult)
                psum_t = ptpool.tile([128, 128], F32, tag="psum_t")
                nc.tensor.transpose(psum_t[:], sent_g[:], ident_sb[:])
                nc.vector.tensor_copy(
                    out=sent_t[:, gstart: gstart + gsize],
                    in_=psum_t[:, :gsize])
                nc.tensor.matmul(psum_hid[:, gstart: gstart + gsize],
                                 lhsT=w_hid_sb[:],
                                 rhs=sent_t[:, gstart: gstart + gsize],
                                 start=True, stop=True)
                nc.scalar.activation(
                    out=hid[:, gstart: gstart + gsize],
                    in_=psum_hid[:, gstart: gstart + gsize],
                    func=mybir.ActivationFunctionType.Tanh,
                    bias=b_hid_sb[:, 0:1])
                # head + BCE partials for this group (hidden under gathers)
                gsl = slice(gstart, gstart + gsize)
                nc.tensor.matmul(psum_p[:, gsl], lhsT=w_out_sb[:],
                                 rhs=hid[:, gsl], start=True, stop=True)
                nc.scalar.activation(
                    out=ep[:, gsl], in_=psum_p[:, gsl],
                    func=mybir.ActivationFunctionType.Exp,
                    bias=b_out_sb[0:1, 0:1])
                nc.scalar.activation(
                    out=sp[:, gsl], in_=ep[:, gsl],
                    func=mybir.ActivationFunctionType.Ln,
                    bias=1.0, accum_out=sp_sums[:, g: g + 1])
                nc.vector.tensor_scalar(
                    out=x_sb[:, gsl], in0=psum_p[:, gsl],
                    scalar1=b_out_sb[0:1, 0:1],
                    scalar2=None, op0=mybir.AluOpType.add)
                nc.vector.tensor_tensor(out=yx[:, gsl], in0=y_sb[:, gsl],
                                        in1=x_sb[:, gsl],
                                        op=mybir.AluOpType.mult)

            # ---- final reduction tail ----
            yx_sum = mpool.tile([1, 1], F32)
            nc.vector.tensor_reduce(out=yx_sum[:], in_=yx[:],
                                    axis=mybir.AxisListType.X,
                                    op=mybir.AluOpType.add)
            sp_tot = mpool.tile([1, 1], F32)
            nc.vector.tensor_reduce(out=sp_tot[:], in_=sp_sums[:],
                                    axis=mybir.AxisListType.X,
                                    op=mybir.AluOpType.add)
            loss = mpool.tile([1, 1], F32)
            nc.vector.tensor_tensor(out=loss[:], in0=sp_tot[:], in1=yx_sum[:],
                                    op=mybir.AluOpType.subtract)
            nc.sync.dma_start(out=out_d[:], in_=loss[:])

    nc.compile()
    return nc


def _prep_inputs(token_ids, segment_ids, y_true, embed_table, W_hid, b_hid,
                 W_out, b_out):
    token_ids = np.asarray(token_ids, dtype=np.int64)
    segment_ids = np.asarray(segment_ids, dtype=np.int64)
    y_true = np.asarray(y_true, dtype=np.float32)
    embed_bf16 = np.ascontiguousarray(
        np.asarray(embed_table, dtype=np.float32).astype(BF16_NP))

    bounds = np.searchsorted(segment_ids, np.arange(0, B + 1, SEGS_PER_CORE))
    counts = np.bincount(segment_ids, minlength=B).astype(np.float32)
    recip_all = 1.0 / np.maximum(counts, 1.0)

    starts = np.asarray(GROUP_STARTS, dtype=np.int64)
    n_buckets = N_GROUPS * N_QUARTERS

    # per (core, bucket): loc-sorted-by-(band, vocab) indices + seg + band
    per_core = []
    for c in range(N_CORES):
        lo, hi = bounds[c], bounds[c + 1]
        tid = token_ids[lo:hi]
        seg_loc = segment_ids[lo:hi] - c * SEGS_PER_CORE
        grp = np.searchsorted(starts[1:], seg_loc, side="right")
        sig = seg_loc - starts[grp]
        band = sig // BAND
        q = tid // VQ
        locv = tid - q * VQ
        order = np.lexsort((locv, band, q, grp))
        grp_s, q_s = grp[order], q[order]
        li_s, sg_s, bd_s = locv[order], sig[order], band[order]
        bucket = grp_s * N_QUARTERS + q_s
        bnd = np.searchsorted(bucket, np.arange(n_buckets + 1))
        subs = []
        for sb in range(n_buckets):
            s, e = bnd[sb], bnd[sb + 1]
            subs.append((li_s[s:e].astype(np.int64),
                         sg_s[s:e].astype(np.float32),
                         bd_s[s:e].astype(np.int64)))
        per_core.append(subs)

    # uniform tile counts across cores (one gather call per bucket keeps
    # the queue rotation strict regardless of per-quarter tile counts) +
    # band->tile ranges (union over cores)
    tiles = []
    band_ranges = []
    for sb in range(n_buckets):
        t_gq = max(max((len(per_core[c][sb][0]) + 127) // 128, 1)
                   for c in range(N_CORES))
        tiles.append(t_gq)
        g = sb // N_QUARTERS
        nb = (GROUP_SEGS[g] + BAND - 1) // BAND
        rng = {}
        for c in range(N_CORES):
            bd = per_core[c][sb][2]
            for b in range(nb):
                pos = np.flatnonzero(bd == b)
                if pos.size == 0:
                    continue
                lo_t, hi_t = pos[0] // 128, pos[-1] // 128 + 1
                if b in rng:
                    rng[b] = (min(rng[b][0], lo_t), max(rng[b][1], hi_t))
                else:
                    rng[b] = (lo_t, hi_t)
        band_ranges.append(tuple((b, lo, hi) for b, (lo, hi)
                                 in sorted(rng.items())))
    struct = tuple((tiles[sb], band_ranges[sb]) for sb in range(n_buckets))

    tile_off = np.cumsum([0] + tiles).tolist()
    col16 = [t * 8 for t in tiles]
    col16_off = np.cumsum([0] + col16).tolist()
    tot_tiles = tile_off[-1]

    iota = np.broadcast_to(np.arange(128, dtype=np.float32),
                           (128, 128)).astype(BF16_NP)
    in_maps = []
    for c in range(N_CORES):
        idx_arr = np.zeros((128, col16_off[-1]), dtype=np.int16)
        seg_arr = np.full((128, tot_tiles), -1.0, dtype=BF16_NP)
        for sb in range(n_buckets):
            li, sg, _ = per_core[c][sb]
            n = li.shape[0]
            nidx = tiles[sb] * 128
            ip = np.zeros(nidx, dtype=np.int16)
            ip[:n] = li
            if n:
                ip[n:] = li[-1]          # pad: repeat last row (seg=-1)
            sp = np.full(nidx, -1.0, dtype=np.float32)
            sp[:n] = sg
            wrapped = ip.reshape(nidx // 16, 16).T
            idx_arr[:, col16_off[sb]: col16_off[sb + 1]] = np.tile(
                wrapped, (8, 1))
            seg_arr[:, tile_off[sb]: tile_off[sb + 1]] = (
                sp.reshape(tiles[sb], 128).T.astype(BF16_NP))
        recip_c = np.ones((128, N_GROUPS), dtype=np.float32)
        for g in range(N_GROUPS):
            gstart, gsize = GROUP_STARTS[g], GROUP_SEGS[g]
            recip_c[:gsize, g] = recip_all[
                c * SEGS_PER_CORE + gstart: c * SEGS_PER_CORE + gstart + gsize]
        in_maps.append({
            "embed": embed_bf16,
            "idx": idx_arr,
            "seg": seg_arr,
            "recip": recip_c,
            "iota": iota,
            "ident": np.eye(128, dtype=np.float32),
            "y": np.ascontiguousarray(
                y_true[c * SEGS_PER_CORE:(c + 1) * SEGS_PER_CORE]
            ).reshape(1, SEGS_PER_CORE),
            "w_hid": np.ascontiguousarray(np.asarray(W_hid, dtype=np.float32)),
            "b_hid": np.asarray(b_hid, dtype=np.float32).reshape(H, 1),
            "w_out": np.ascontiguousarray(np.asarray(W_out, dtype=np.float32)),
            "b_out": np.asarray(b_out, dtype=np.float32).reshape(1, 1),
        })
    return struct, in_maps


_CACHE = {}


def _get_nc(struct):
    nc = _CACHE.get(struct)
    if nc is None:
        nc = bacc.Bacc("TRN2", target_bir_lowering=False, debug=False,
                       num_devices=N_CORES, num_swdge_queues=N_QUARTERS)
        _build(nc, struct)
        _CACHE[struct] = nc
    return nc


def kernel(token_ids, segment_ids, y_true, embed_table, W_hid, b_hid, W_out,
           b_out, _trace=False, _trace_kwargs=None):
    struct, in_maps = _prep_inputs(token_ids, segment_ids, y_true,
                                   embed_table, W_hid, b_hid, W_out, b_out)
    nc = _get_nc(struct)
    res = run_bass_kernel_spmd(nc, in_maps, core_ids=list(range(N_CORES)),
                               trace=_trace, **(_trace_kwargs or {}))
    total = np.float64(0.0)
    for r in res.results:
        total += np.float64(r["out"][0, 0])
    out = np.array(np.float32(total))
    if _trace:
        return out, res
    return out


# revision 10
# speedup vs baseline: 1.0101x; 1.0051x over previous
"""Trainium2 Bass kernel for the DAN classifier (gather + segment-mean + MLP + BCE).

Data-parallel over 8 cores (512 sentences each). Host sorts each core's
tokens by (96-seg group, vocab-quarter, 32-seg band, vocab) and pads each
(group, quarter) bucket to a shared tile count. On device, one dma_gather
per bucket streams bf16 embedding rows over 4 SWDGE queues (emission
strictly rotates queues so Tile's round-robin DMASW lanes stay
queue-locked). Segment sums accumulate on the TensorEngine via narrow
32-seg one-hot matmuls (host-known band->tile ranges) into one PSUM
accumulation group per seg-group, bracketed by full-width start/stop
matmuls; quarters accumulate into the same bank. MLP head + BCE on-chip;
host sums the 8 partial losses.
"""

import sys

try:
    import concourse  # noqa: F401
except ImportError:
    sys.path.insert(0, "/opt/trn_rl_repo")

import ml_dtypes
import numpy as np

import concourse.tile as tile
from concourse import bacc, mybir
from concourse.bass_utils import run_bass_kernel_spmd

V = 100000
H = 128
B = 4096
T = 409600
N_CORES = 8

SEGS_PER_CORE = B // N_CORES          # 512
GROUP_SEGS = (96, 96, 96, 96, 96, 32)
GROUP_STARTS = (0, 96, 192, 288, 384, 480)
N_GROUPS = len(GROUP_SEGS)
N_QUARTERS = 4
VQ = 25600
BAND = 32

F32 = mybir.dt.float32
BF16 = mybir.dt.bfloat16
I16 = mybir.dt.int16
BF16_NP = ml_dtypes.bfloat16


def _build(nc, struct):
    """struct: tuple of per-bucket (t_gq, ((band, lo, hi), ...)) in
    (g, q) order; all compile-time constants shared across cores."""
    n_buckets = N_GROUPS * N_QUARTERS
    tiles = [s[0] for s in struct]
    t_max = max(tiles)
    tot_tiles = sum(tiles)
    col16 = [t * 8 for t in tiles]          # idx cols (128 idx = 8 cols)
    col16_off = np.cumsum([0] + col16).tolist()
    tile_off = np.cumsum([0] + tiles).tolist()

    embed = nc.dram_tensor("embed", [V, H], BF16, kind="ExternalInput")
    idx_d = nc.dram_tensor("idx", [128, col16_off[-1]], I16,
                           kind="ExternalInput")
    seg_d = nc.dram_tensor("seg", [128, tot_tiles], BF16,
                           kind="ExternalInput")
    recip_d = nc.dram_tensor("recip", [128, N_GROUPS], F32,
                             kind="ExternalInput")
    iota_d = nc.dram_tensor("iota", [128, 128], BF16, kind="ExternalInput")
    ident_d = nc.dram_tensor("ident", [128, 128], F32, kind="ExternalInput")
    y_d = nc.dram_tensor("y", [1, SEGS_PER_CORE], F32, kind="ExternalInput")
    w_hid_d = nc.dram_tensor("w_hid", [H, H], F32, kind="ExternalInput")
    b_hid_d = nc.dram_tensor("b_hid", [H, 1], F32, kind="ExternalInput")
    w_out_d = nc.dram_tensor("w_out", [H, 1], F32, kind="ExternalInput")
    b_out_d = nc.dram_tensor("b_out", [1, 1], F32, kind="ExternalInput")
    out_d = nc.dram_tensor("out", [1, 1], F32, kind="ExternalOutput")

    with tile.TileContext(nc) as tc:
        with (
            tc.tile_pool(name="const", bufs=1) as cpool,
            tc.tile_pool(name="gather", bufs=4) as gpool,
            tc.tile_pool(name="onehot", bufs=4) as opool,
            tc.tile_pool(name="mlp", bufs=1) as mpool,
            tc.tile_pool(name="psum_seg", bufs=3, space="PSUM") as pspool,
            tc.tile_pool(name="psum_t", bufs=2, space="PSUM") as ptpool,
            tc.tile_pool(name="psum_mlp", bufs=1, space="PSUM") as pmpool,
        ):
            # ---- idx loads: group 0 in one transfer, rest in another ----
            idx_sb = cpool.tile([128, col16_off[-1]], I16)
            g0_hi = col16_off[N_QUARTERS]
            nc.sync.dma_start(out=idx_sb[:, :g0_hi], in_=idx_d[:, :g0_hi])
            nc.sync.dma_start(out=idx_sb[:, g0_hi:], in_=idx_d[:, g0_hi:])
            seg_sb = cpool.tile([128, tot_tiles], BF16)
            nc.scalar.dma_start(out=seg_sb[:], in_=seg_d[:])
            iota_sb = cpool.tile([128, 128], BF16)
            nc.scalar.dma_start(out=iota_sb[:], in_=iota_d[:])
            recip_sb = cpool.tile([128, N_GROUPS], F32)
            nc.scalar.dma_start(out=recip_sb[:], in_=recip_d[:])
            ident_sb = cpool.tile([128, 128], F32)
            nc.scalar.dma_start(out=ident_sb[:], in_=ident_d[:])
            y_sb = cpool.tile([1, SEGS_PER_CORE], F32)
            nc.scalar.dma_start(out=y_sb[:], in_=y_d[:])
            w_hid_sb = cpool.tile([H, H], F32)
            nc.scalar.dma_start(out=w_hid_sb[:], in_=w_hid_d[:])
            b_hid_sb = cpool.tile([H, 1], F32)
            nc.scalar.dma_start(out=b_hid_sb[:], in_=b_hid_d[:])
            w_out_sb = cpool.tile([H, 1], F32)
            nc.scalar.dma_start(out=w_out_sb[:], in_=w_out_d[:])
            b_out_sb = cpool.tile([1, 1], F32)
            nc.scalar.dma_start(out=b_out_sb[:], in_=b_out_d[:])

            # ---- gather preps (pipelined: gen overlaps previous DMA) ----
            # sub-calls of <=1024 descriptors to stay within the SWDGE
            # descriptor-ring carveout (prepared descs can't be reclaimed
            # until triggered). Emission strictly rotates queues 0..3 so the
            # Tile scheduler's round-robin DMASW lane assignment (8 lanes)
            # keeps each lane locked to a single queue; tile counts are
            # equalized across quarters per group to preserve the rotation.
            gts = {}
            for g in range(N_GROUPS):
                for q in range(N_QUARTERS):
                    sb = g * N_QUARTERS + q
                    gt = gpool.tile([128, t_max, H], BF16, tag=f"g{q}")
                    gts[sb] = gt
                for q in range(N_QUARTERS):
                    sb = g * N_QUARTERS + q
                    t_gq = tiles[sb]
                    nidx = t_gq * 128
                    c16 = col16_off[sb]
                    nc.gpsimd.dma_gather(
                        gts[sb][:, :t_gq, :],
                        embed[q * VQ: q * VQ + min(VQ, V - q * VQ), :],
                        idx_sb[:, c16: c16 + nidx // 16],
                        nidx, nidx, H,
                        single_packet=False, queue_num=q)

            # activation table warmups (before first tanh, off critical path)
            warm_act = cpool.tile([1, 1], F32)
            nc.vector.memset(warm_act[:], 0.5)
            for fn in (mybir.ActivationFunctionType.Tanh,
                       mybir.ActivationFunctionType.Exp,
                       mybir.ActivationFunctionType.Ln):
                nc.scalar.activation(out=warm_act[:], in_=warm_act[:], func=fn)

            # ---- per-group segment-sum matmuls + MLP pipeline ----
            sent_t = mpool.tile([128, SEGS_PER_CORE], F32)   # [H, seg]
            psum_hid = pmpool.tile([128, SEGS_PER_CORE], F32, tag="psum_hid")
            hid = mpool.tile([128, SEGS_PER_CORE], F32)
            psum_p = pmpool.tile([1, SEGS_PER_CORE], F32, tag="psum_p")
            ep = mpool.tile([1, SEGS_PER_CORE], F32)
            sp = mpool.tile([1, SEGS_PER_CORE], F32)
            sp_sums = mpool.tile([1, N_GROUPS], F32)
            x_sb = mpool.tile([1, SEGS_PER_CORE], F32)
            yx = mpool.tile([1, SEGS_PER_CORE], F32)

            for g in range(N_GROUPS):
                psum_g = pspool.tile([128, H], F32, tag="seg")
                # One PSUM accumulation group per seg-group: the first matmul
                # (q=0, tile 0) uses a full 128-wide one-hot with start=True
                # (zeroes every partition of the bank region); all later
                # matmuls accumulate 32-seg band slices; the global last one
                # stops the group. PE is in-order, so the sequence is exact.
                t_q3 = tiles[g * N_QUARTERS + 3]

                def _full_mm(q, j, start, stop):
                    sb_ = g * N_QUARTERS + q
                    ohf = opool.tile([128, 1, 128], BF16, tag="ohf")
                    c0 = tile_off[sb_] + j
                    nc.vector.tensor_tensor(
                        out=ohf[:],
                        in0=seg_sb[:, c0: c0 + 1]
                        .rearrange("p (t u) -> p t u", u=1)
                        .to_broadcast([128, 1, 128]),
                        in1=iota_sb[:]
                        .rearrange("p (u m) -> p u m", u=1)
                        .to_broadcast([128, 1, 128]),
                        op=mybir.AluOpType.is_equal,
                    )
                    nc.tensor.matmul(
                        psum_g[:], lhsT=ohf[:, 0, :], rhs=gts[sb_][:, j, :],
                        start=start, stop=stop)

                def _skip(q, j):
                    return (q == 0 and j == 0) or (q == 3 and j == t_q3 - 1)

                # mm0: full-width start (zeroes the whole bank region)
                _full_mm(0, 0, True, False)
                for q in range(N_QUARTERS):
                    sb = g * N_QUARTERS + q
                    gt = gts[sb]
                    # one one-hot batch per (bucket, band) tile range
                    ohs = {}
                    for (b, lo, hi) in struct[sb][1]:
                        nt = hi - lo
                        oh = opool.tile([128, t_max, BAND], BF16, tag="oh")
                        c0 = tile_off[sb] + lo
                        nc.vector.tensor_tensor(
                            out=oh[:, :nt, :],
                            in0=seg_sb[:, c0: c0 + nt]
                            .rearrange("p (t u) -> p t u", u=1)
                            .to_broadcast([128, nt, BAND]),
                            in1=iota_sb[:, b * BAND: (b + 1) * BAND]
                            .rearrange("p (u m) -> p u m", u=1)
                            .to_broadcast([128, nt, BAND]),
                            op=mybir.AluOpType.is_equal,
                        )
                        ohs[b] = (oh, lo)
                    for (b, lo, hi) in struct[sb][1]:
                        oh, lo0 = ohs[b]
                        for j in range(lo, hi):
                            if _skip(q, j):
                                continue
                            nc.tensor.matmul(
                                psum_g[b * BAND: (b + 1) * BAND, :],
                                lhsT=oh[:, j - lo0, :],
                                rhs=gt[:, j, :],
                                start=False, stop=False)
                # mmN: full-width stop (closes the group over the full region)
                _full_mm(3, t_q3 - 1, False, True)

                # segment means for this group
                gstart, gsize = GROUP_STARTS[g], GROUP_SEGS[g]
                sent_g = mpool.tile([128, H], F32, tag="sent_g")
                nc.vector.tensor_scalar(
                    out=sent_g[:gsize, :], in0=psum_g[:gsize, :],
                    scalar1=recip_sb[:gsize, g: g + 1], scalar2=None,
                    op0=mybir.AluOpType.mult)
                psum_t = ptpool.tile([128, 128], F32, tag="psum_t")
                nc.tensor.transpose(psum_t[:], sent_g[:], ident_sb[:])
                nc.vector.tensor_copy(
                    out=sent_t[:, gstart: gstart + gsize],
                    in_=psum_t[:, :gsize])
                nc.tensor.matmul(psum_hid[:, gstart: gstart + gsize],
                                 lhsT=w_hid_sb[:],
                                 rhs=sent_t[:, gstart: gstart + gsize],
                                 start=True, stop=True)
                nc.scalar.activation(
                    out=hid[:, gstart: gstart + gsize],
                    in_=psum_hid[:, gstart: gstart + gsize],
                    func=mybir.ActivationFunctionType.Tanh,
                    bias=b_hid_sb[:, 0:1])
                # head + BCE partials for this group (hidden under gathers)
                gsl = slice(gstart, gstart + gsize)
                nc.tensor.matmul(psum_p[:, gsl], lhsT=w_out_sb[:],
                                 rhs=hid[:, gsl], start=True, stop=True)
                nc.scalar.activation(
                    out=ep[:, gsl], in_=psum_p[:, gsl],
                    func=mybir.ActivationFunctionType.Exp,
                    bias=b_out_sb[0:1, 0:1])
                nc.scalar.activation(
                    out=sp[:, gsl], in_=ep[:, gsl],
                    func=mybir.ActivationFunctionType.Ln,
                    bias=1.0, accum_out=sp_sums[:, g: g + 1])
                nc.vector.tensor_scalar(
                    out=x_sb[:, gsl], in0=psum_p[:, gsl],
                    scalar1=b_out_sb[0:1, 0:1],
                    scalar2=None, op0=mybir.AluOpType.add)
                nc.vector.tensor_tensor(out=yx[:, gsl], in0=y_sb[:, gsl],
                                        in1=x_sb[:, gsl],
                                        op=mybir.AluOpType.mult)

            # ---- final reduction tail ----
            yx_sum = mpool.tile([1, 1], F32)
            nc.vector.tensor_reduce(out=yx_sum[:], in_=yx[:],
                                    axis=mybir.AxisListType.X,
                                    op=mybir.AluOpType.add)
            sp_tot = mpool.tile([1, 1], F32)
            nc.vector.tensor_reduce(out=sp_tot[:], in_=sp_sums[:],
                                    axis=mybir.AxisListType.X,
                                    op=mybir.AluOpType.add)
            loss = mpool.tile([1, 1], F32)
            nc.vector.tensor_tensor(out=loss[:], in0=sp_tot[:], in1=yx_sum[:],
                                    op=mybir.AluOpType.subtract)
            nc.sync.dma_start(out=out_d[:], in_=loss[:])

    nc.compile()
    return nc


def _prep_inputs(token_ids, segment_ids, y_true, embed_table, W_hid, b_hid,
                 W_out, b_out):
    token_ids = np.asarray(token_ids, dtype=np.int64)
    segment_ids = np.asarray(segment_ids, dtype=np.int64)
    y_true = np.asarray(y_true, dtype=np.float32)
    embed_bf16 = np.ascontiguousarray(
        np.asarray(embed_table, dtype=np.float32).astype(BF16_NP))

    bounds = np.searchsorted(segment_ids, np.arange(0, B + 1, SEGS_PER_CORE))
    counts = np.bincount(segment_ids, minlength=B).astype(np.float32)
    recip_all = 1.0 / np.maximum(counts, 1.0)

    starts = np.asarray(GROUP_STARTS, dtype=np.int64)
    n_buckets = N_GROUPS * N_QUARTERS

    # per (core, bucket): loc-sorted-by-(band, vocab) indices + seg + band
    per_core = []
    for c in range(N_CORES):
        lo, hi = bounds[c], bounds[c + 1]
        tid = token_ids[lo:hi]
        seg_loc = segment_ids[lo:hi] - c * SEGS_PER_CORE
        grp = np.searchsorted(starts[1:], seg_loc, side="right")
        sig = seg_loc - starts[grp]
        band = sig // BAND
        q = tid // VQ
        locv = tid - q * VQ
        order = np.lexsort((locv, band, q, grp))
        grp_s, q_s = grp[order], q[order]
        li_s, sg_s, bd_s = locv[order], sig[order], band[order]
        bucket = grp_s * N_QUARTERS + q_s
        bnd = np.searchsorted(bucket, np.arange(n_buckets + 1))
        subs = []
        for sb in range(n_buckets):
            s, e = bnd[sb], bnd[sb + 1]
            subs.append((li_s[s:e].astype(np.int64),
                         sg_s[s:e].astype(np.float32),
                         bd_s[s:e].astype(np.int64)))
        per_core.append(subs)

    # tile counts equalized across cores AND quarters per group: each
    # wave completes when its slowest queue drains, so balancing the four
    # buckets' descriptor counts (+~3% padding) beats unbalanced waves
    # under the per-queue in-flight ceiling. + band->tile ranges (union
    # over cores)
    t_group = []
    for g in range(N_GROUPS):
        t_group.append(max(
            max((len(per_core[c][g * N_QUARTERS + q][0]) + 127) // 128, 1)
            for c in range(N_CORES) for q in range(N_QUARTERS)))
    tiles = []
    band_ranges = []
    for sb in range(n_buckets):
        t_gq = t_group[sb // N_QUARTERS]
        tiles.append(t_gq)
        g = sb // N_QUARTERS
        nb = (GROUP_SEGS[g] + BAND - 1) // BAND
        rng = {}
        for c in range(N_CORES):
            bd = per_core[c][sb][2]
            for b in range(nb):
                pos = np.flatnonzero(bd == b)
                if pos.size == 0:
                    continue
                lo_t, hi_t = pos[0] // 128, pos[-1] // 128 + 1
                if b in rng:
                    rng[b] = (min(rng[b][0], lo_t), max(rng[b][1], hi_t))
                else:
                    rng[b] = (lo_t, hi_t)
        band_ranges.append(tuple((b, lo, hi) for b, (lo, hi)
                                 in sorted(rng.items())))
    struct = tuple((tiles[sb], band_ranges[sb]) for sb in range(n_buckets))

    tile_off = np.cumsum([0] + tiles).tolist()
    col16 = [t * 8 for t in tiles]
    col16_off = np.cumsum([0] + col16).tolist()
    tot_tiles = tile_off[-1]

    iota = np.broadcast_to(np.arange(128, dtype=np.float32),
                           (128, 128)).astype(BF16_NP)
    in_maps = []
    for c in range(N_CORES):
        idx_arr = np.zeros((128, col16_off[-1]), dtype=np.int16)
        seg_arr = np.full((128, tot_tiles), -1.0, dtype=BF16_NP)
        for sb in range(n_buckets):
            li, sg, _ = per_core[c][sb]
            n = li.shape[0]
            nidx = tiles[sb] * 128
            ip = np.zeros(nidx, dtype=np.int16)
            ip[:n] = li
            if n:
                ip[n:] = li[-1]          # pad: repeat last row (seg=-1)
            sp = np.full(nidx, -1.0, dtype=np.float32)
            sp[:n] = sg
            wrapped = ip.reshape(nidx // 16, 16).T
            idx_arr[:, col16_off[sb]: col16_off[sb + 1]] = np.tile(
                wrapped, (8, 1))
            seg_arr[:, tile_off[sb]: tile_off[sb + 1]] = (
                sp.reshape(tiles[sb], 128).T.astype(BF16_NP))
        recip_c = np.ones((128, N_GROUPS), dtype=np.float32)
        for g in range(N_GROUPS):
            gstart, gsize = GROUP_STARTS[g], GROUP_SEGS[g]
            recip_c[:gsize, g] = recip_all[
                c * SEGS_PER_CORE + gstart: c * SEGS_PER_CORE + gstart + gsize]
        in_maps.append({
            "embed": embed_bf16,
            "idx": idx_arr,
            "seg": seg_arr,
            "recip": recip_c,
            "iota": iota,
            "ident": np.eye(128, dtype=np.float32),
            "y": np.ascontiguousarray(
                y_true[c * SEGS_PER_CORE:(c + 1) * SEGS_PER_CORE]
            ).reshape(1, SEGS_PER_CORE),
            "w_hid": np.ascontiguousarray(np.asarray(W_hid, dtype=np.float32)),
            "b_hid": np.asarray(b_hid, dtype=np.float32).reshape(H, 1),
            "w_out": np.ascontiguousarray(np.asarray(W_out, dtype=np.float32)),
            "b_out": np.asarray(b_out, dtype=np.float32).reshape(1, 1),
        })
    return struct, in_maps


_CACHE = {}


def _get_nc(struct):
    nc = _CACHE.get(struct)
    if nc is None:
        nc = bacc.Bacc("TRN2", target_bir_lowering=False, debug=False,
                       num_devices=N_CORES, num_swdge_queues=N_QUARTERS)
        _build(nc, struct)
        _CACHE[struct] = nc
    return nc


def kernel(token_ids, segment_ids, y_true, embed_table, W_hid, b_hid, W_out,
           b_out, _trace=False, _trace_kwargs=None):
    struct, in_maps = _prep_inputs(token_ids, segment_ids, y_true,
                                   embed_table, W_hid, b_hid, W_out, b_out)
    nc = _get_nc(struct)
    res = run_bass_kernel_spmd(nc, in_maps, core_ids=list(range(N_CORES)),
                               trace=_trace, **(_trace_kwargs or {}))
    total = np.float64(0.0)
    for r in res.results:
        total += np.float64(r["out"][0, 0])
    out = np.array(np.float32(total))
    if _trace:
        return out, res
    return out
